# revision 25
# baseline (speedup 1.0000x reference)
"""Trainium2 Bass kernel for nn_Attention_47467978555850.

Multi-head attention (B=8, N=1024, E=768, H=12, D=64), fp32 in/out.
Sharding: data-parallel over batch - one batch element per NeuronCore (8 cores),
no collectives.

All matmul operands are fp16 (host-cast inputs; fp16 mantissa error 2^-11 is
below the fp32r matmul error of the original fp32 pipeline, and every value
here is far inside fp16 range). PSUM accumulation is fp32 throughout. fp16
also enables fast weight loads (FWL) and halves input DMA.

Per-core dataflow (everything stays in "transposed" space so no on-device
transposes are needed; the host transposes/casts x and y, costing no HW time):

  xTh [E, N] fp16 --(w_qkv lhsT-stationary)--> qT, kT fp16 [head-dim major, N]
              (2 heads packed per 128-partition tile)
  xTh (stationary) x w_v (moving) -> v [N, d] -> v_aug fp16 [N, H*128],
              each head block = [v(64) | ones(64)]
  For each head pair f = (2f, 2f+1), for each 512-wide i-chunk c:
    S^T[j,i] both heads : row-packed K=64 matmuls into the two banks of
              one [128,1024] PSUM tile (head A rows 0-63 -> cols 0:512, head B
              rows 64-127 -> cols 512:1024) - concurrent PE row groups
    E = Exp(S^T/8) fp16 : one [128,1024] ScalarE op per (c, j); ScalarE is the
              attention-phase critical engine (~1.07us/op, 96 ops)
    acc[:,0:512]   += vaugA^T @ E[:,0:512]    (8-deep fp16 chains; rows 0-63 =
    acc[:,512:1024]+= vaugB^T @ E[:,512:1024]  out, 64-127 = denominator)
    outT fp16 = acc[0:64] * reciprocal_approx_fast(denominator)
              (the custom DVE op breaks at base_partition 64, so the
              replicated denominator rows are first copied down to 0-63)
  yT = w_projh^T @ outT + b  ->  DMA out as yT [E, N] fp32

PSUM: ONE pool of 4x [128,1024] tiles (8 banks) shared by S / acc / qk-feat /
v / proj so the next pair's feature matmuls can fill PE slack while ScalarE
crunches exp. Next-pair q/k feat tiles are emitted between attention chunks.

Measured absmax-rel error vs fp64: ~4e-4 (gate 2e-2).
"""

import numpy as np
import ml_dtypes

B, N, E = 8, 1024, 768
H, D = 12, 64
NE = E // 128        # 6  e-tiles
NT = N // 128        # 8  token tiles
JT = N // 128        # 8  j tiles (attention context)
CH = N // 512        # 2  512-wide moving chunks
DA = 2 * D           # 128 cols/head in v_aug: [v(64), ones(64)] — the
                     # ones block makes the attn@v matmul replicate the
                     # softmax denominator across 64 psum partitions

_NC_CACHE = {}


def _emit(tc, pools, aps, dbg=None):
    import concourse.mybir as mybir

    nc = tc.nc
    f32 = mybir.dt.float32
    fp16 = mybir.dt.float16
    consts, wstr, expp, qkp, rbp, ytp, psu = pools
    xT, w_qkv, w_projh, b_proj, yT = aps

    def dump(name, src):
        # debug-only: copy an SBUF/PSUM AP out to a DRAM tensor
        if dbg is not None and name in dbg:
            nc.sync.dma_start(out=dbg[name], in_=src)

    def dump_psum(name, src, dt):
        if dbg is not None and name in dbg:
            t = consts.tile(list(src.shape), dt, tag=f"dbg{name}", name=f"dbg{name}")
            nc.vector.tensor_copy(out=t, in_=src)
            nc.sync.dma_start(out=dbg[name], in_=t)

    # ---- persistent SBUF tiles ----
    xt = [consts.tile([128, N], fp16, tag=f"xt{e}", name=f"xt{e}") for e in range(NE)]
    wv = [consts.tile([128, E], fp16, tag=f"wv{e}", name=f"wv{e}") for e in range(NE)]
    b_sb = consts.tile([128, NE], f32, tag="b_sb", name="b_sb")
    vaug = [consts.tile([128, H * DA], fp16, tag=f"va{t}", name=f"va{t}")
            for t in range(NT)]
    outT = [consts.tile([128, N], fp16, tag=f"oT{e}", name=f"oT{e}") for e in range(NE)]

    def load_w_tiles(fcol, fname):
        # all six 128x128 e-blocks of one feature column in a single DMA
        w = wstr.tile([128, E], fp16, tag="w", name=f"w{fname}")
        nc.sync.dma_start(
            out=w.rearrange("p (e c) -> p e c", e=NE),
            in_=w_qkv[:, fcol:fcol + 128].rearrange("(e p) c -> p e c", p=128))
        return [w[:, e * 128:(e + 1) * 128] for e in range(NE)]

    w_q0 = load_w_tiles(0, "q0")
    for e in range(NE):
        nc.sync.dma_start(out=xt[e], in_=xT[e * 128:(e + 1) * 128, :])
    w_k0 = load_w_tiles(E, "k0")
    w_q1 = load_w_tiles(128, "q1")
    w_k1 = load_w_tiles(E + 128, "k1")
    nc.sync.dma_start(out=b_sb, in_=b_proj.rearrange("(t p) -> p t", p=128))

    # PE warmup: dummy matmuls on zeroed SBUF spanning the input-DMA window,
    # so the HAM clock gate reaches (and keeps) 2.4 GHz before the first real
    # matmul; ~8 run cold then ~36 warm, covering ~12 us
    wu = wstr.tile([128, E], fp16, tag="w", name="warmup_w")
    nc.vector.memset(wu, 0.0)
    ps_wu = psu.tile([128, N], f32, tag="ps", name="ps_warmup")
    for i in range(44):
        nc.tensor.matmul(out=ps_wu[:, 0:512], lhsT=wu[:, 0:128],
                         rhs=wu[:, 0:512], start=True, stop=True)

    def load_wv():
        for e in range(NE):
            nc.sync.dma_start(out=wv[e],
                              in_=w_qkv[e * 128:(e + 1) * 128, 2 * E:3 * E])

    # ---- filler queue: generators that emit ~2 matmuls per pull, used to
    # spread feat/proj work into the exp-paced gaps inside attention chunks
    fillq = []

    def fill(n=1):
        for _ in range(n):
            while fillq:
                try:
                    next(fillq[0][1])
                    return
                except StopIteration:
                    fillq.pop(0)
            return

    def drain(upto_pair):
        while fillq and fillq[0][0] <= upto_pair:
            pair, gen = fillq.pop(0)
            for _ in gen:
                pass

    # ---- q/k feature tiles: w_qkv column block stationary, xT moving ----
    def qk_feat_tile(fcol, fname, wts=None):
        dst = qkp.tile([128, N], fp16, tag="qk", name=f"qk{fname}")
        if wts is None:
            wts = load_w_tiles(fcol, fname)

        def gen():
            ps_qk = psu.tile([128, N], f32, tag="ps", name=f"psqk{fname}")
            for c in range(CH):
                cs = slice(c * 512, (c + 1) * 512)
                for e in range(NE):
                    nc.tensor.matmul(
                        out=ps_qk[:, cs], lhsT=(wts[e]), rhs=(xt[e][:, cs]),
                        start=(e == 0), stop=(e == NE - 1),
                    )
                    if e % 2 == 1:
                        yield
                nc.vector.tensor_copy(out=dst[:, cs], in_=ps_qk[:, cs])
                yield
        return dst, gen()

    # ---- v = x @ w_v  (xT tiles stationary, w_v moving) -> vaug bf16 ----
    def emit_v_phase():
        for t in range(NT):
            ps_v = psu.tile([128, N], f32, tag="ps", name=f"psv{t}")
            for (c0, cl) in ((0, 512), (512, 256)):
                for e in range(NE):
                    nc.tensor.matmul(
                        out=ps_v[:, c0:c0 + cl],
                        lhsT=(xt[e][:, t * 128:(t + 1) * 128]),
                        rhs=(wv[e][:, c0:c0 + cl]),
                        start=(e == 0), stop=(e == NE - 1),
                    )
            va3 = vaug[t].rearrange("p (h c) -> p h c", h=H)
            nc.vector.tensor_copy(
                out=va3[:, :, 0:D],
                in_=ps_v[:, 0:E].rearrange("p (h c) -> p h c", h=H),
            )
            nc.vector.memset(va3[:, :, D:DA], 1.0)
            if t == 0:
                dump("vaug0", vaug[0])

    # ---- attention for head pair f, one 512-wide i-chunk ----
    def attention_chunk(f, c, qTf, kTf, flat=None):
        hA, hB = 2 * f, 2 * f + 1
        cs = slice(c * 512, (c + 1) * 512)

        def mm2exp(j):
            js = slice(j * 128, (j + 1) * 128)
            S = psu.tile([128, N], f32, tag="ps", name=f"S{f}_{c}_{j}")
            for pb, col0 in ((0, 0), (64, 512)):
                nc.tensor.matmul(
                    out=S[:, col0:col0 + 512],
                    lhsT=kTf[pb:pb + 64, js],
                    rhs=qTf[pb:pb + 64, cs],
                    start=True, stop=True,
                )
            Ej = expp.tile([128, N], fp16, tag="e", name=f"E{f}_{c}_{j}")
            nc.scalar.activation(
                out=Ej, in_=S,
                func=mybir.ActivationFunctionType.Exp, scale=0.125)
            if f == 0 and c == 0 and j == 0:
                dump("E000", Ej)
            return Ej

        def mm3(j, Ej):
            for col0, h in ((0, hA), (512, hB)):
                nc.tensor.matmul(
                    out=acc[:, col0:col0 + 512],
                    lhsT=(vaug[j][:, h * DA:(h + 1) * DA]),
                    rhs=(Ej[:, col0:col0 + 512]),
                    start=(j == 0), stop=(j == JT - 1),
                )

        if flat == "scores":
            # emit only the score/exp stream; mm3 chains deferred (pair 0)
            Es = [mm2exp(j) for j in range(JT)]
            return Es
        if flat == "mm3":
            acc = psu.tile([128, N], f32, tag="ps", name=f"acc{f}_{c}")
            for j in range(JT):
                fill()
                mm3(j, qTf[j])           # qTf carries the E list here
        else:
            E_cur = mm2exp(0)
            acc = psu.tile([128, N], f32, tag="ps", name=f"acc{f}_{c}")
            for j in range(JT):
                E_next = mm2exp(j + 1) if j + 1 < JT else None
                fill()
                mm3(j, E_cur)
                E_cur = E_next

        if f == 0 and c == 0:
            dump_psum("acc00", acc, f32)
        # custom-DVE ops misbehave at base_partition 64: stage the replicated
        # denominators down to partitions 0-63 with a native copy first
        den = rbp.tile([128, N], f32, tag="den", name=f"den{f}_{c}")
        rb = rbp.tile([128, N], f32, tag="rb", name=f"rb{f}_{c}")
        nc.vector.tensor_copy(out=den[0:64, :], in_=acc[64:128, :])
        nc.vector.reciprocal_approx_fast(out=rb[0:64, :], in_=den[0:64, :])
        if f == 0 and c == 0:
            dump("rb00", rb[:, 0:512])
        for col0, h in ((0, hA), (512, hB)):
            pb = (h % 2) * 64
            nc.vector.tensor_mul(outT[f][pb:pb + 64, cs],
                                 acc[0:64, col0:col0 + 512],
                                 rb[0:64, col0:col0 + 512])

    def load_proj_w(g):
        wg = wstr.tile([128, E], fp16, tag="w", name=f"wp{g}")
        nc.sync.dma_start(
            out=wg.rearrange("p (e c) -> p e c", e=NE),
            in_=w_projh[:, g * 128:(g + 1) * 128].rearrange(
                "(e p) c -> p e c", p=128))
        return wg

    # Tile dependencies follow EMISSION order, so a proj matmul reading
    # outT[e] must be emitted after the normalization that writes it.  The
    # e<=4 head of proj(0) can therefore run as filler inside pair 5 (outT[4]
    # is complete once pair 4's norms are emitted), but every e=5 matmul must
    # be emitted after pair 5.
    proj_ps = {}

    def proj_head_gen(g, wg):
        wts = [wg[:, e * 128:(e + 1) * 128] for e in range(NE)]

        def gen():
            ps_y = psu.tile([128, N], f32, tag="ps", name=f"psy{g}")
            proj_ps[g] = ps_y
            for e in range(NE - 1):
                for c in range(CH):
                    cs = slice(c * 512, (c + 1) * 512)
                    nc.tensor.matmul(
                        out=ps_y[:, cs], lhsT=(wts[e]), rhs=(outT[e][:, cs]),
                        start=(e == 0), stop=False,
                    )
                yield
        return gen()

    def proj_finish(g, wg):
        ps_y = proj_ps[g]
        for c in range(CH):
            cs = slice(c * 512, (c + 1) * 512)
            nc.tensor.matmul(
                out=ps_y[:, cs], lhsT=(wg[:, (NE - 1) * 128:NE * 128]),
                rhs=(outT[NE - 1][:, cs]), start=False, stop=True,
            )
        yt = ytp.tile([128, N], f32, tag="yt", name=f"yt{g}")
        nc.scalar.activation(out=yt, in_=ps_y,
                             func=mybir.ActivationFunctionType.Identity,
                             bias=b_sb[:, g:g + 1], scale=1.0)
        nc.sync.dma_start(out=yT[g * 128:(g + 1) * 128, :], in_=yt)

    # ---- main schedule ----
    # prologue: feats for pairs 0 and 1 emitted eagerly (weight tiles were
    # preloaded before the bulk xt DMA).  Pair 0 emits BOTH chunks' score/exp
    # streams back-to-back so ScalarE runs 16 exps while the PE does the whole
    # v phase underneath; its attn@v chains follow the v phase.  From pair 1
    # on, feat tiles for pair f+2 are built as fillers pulled between the
    # j-steps (2 matmuls per pull), so they run inside the exp-paced dribbles
    # instead of as a block at the pair boundary.  The first two proj tiles
    # join the filler queue during pair 5.
    qk = {}
    for fcol, fname, wts in ((0, "q0", w_q0), (E, "k0", w_k0),
                             (128, "q1", w_q1), (E + 128, "k1", w_k1)):
        dst, g = qk_feat_tile(fcol, fname, wts)
        for _ in g:
            pass
        qk.setdefault(fname[1:], []).append(dst)
    qk = {0: tuple(qk["0"]), 1: tuple(qk["1"])}
    dump("q0", qk[0][0])
    dump("k0", qk[0][1])
    load_wv()

    for f in range(NE):
        drain(f)
        qTf, kTf = qk.pop(f)
        if f == 0:
            Es0 = attention_chunk(0, 0, qTf, kTf, flat="scores")
            Es1 = attention_chunk(0, 1, qTf, kTf, flat="scores")
            emit_v_phase()
            attention_chunk(0, 0, Es0, None, flat="mm3")
            attention_chunk(0, 1, Es1, None, flat="mm3")
        else:
            attention_chunk(f, 0, qTf, kTf)
            attention_chunk(f, 1, qTf, kTf)
        if f + 2 < NE:
            dq, gq = qk_feat_tile((f + 2) * 128, f"q{f + 2}")
            dk, gk = qk_feat_tile(E + (f + 2) * 128, f"k{f + 2}")
            qk[f + 2] = (dq, dk)
            fillq.append((f + 2, gq))
            fillq.append((f + 2, gk))
        if f == NE - 3:
            proj_w = [load_proj_w(g) for g in range(NE)]
        if f == NE - 2:
            fillq.append((99, proj_head_gen(0, proj_w[0])))

    dump("outT0", outT[0])

    # ---- proj: yT = w_projh^T @ outT + b  (fp16 x fp16) ----
    drain(99)
    proj_finish(0, proj_w[0])
    for g in range(1, NE):
        for _ in proj_head_gen(g, proj_w[g]):
            pass
        proj_finish(g, proj_w[g])


def build_nc(loop_n=1, debug_dumps=False):
    """Build + compile the per-core Bass program. loop_n>1 wraps the body in a
    dynamic loop (used only for timing runs)."""
    from contextlib import ExitStack
    import concourse.bacc as bacc
    import concourse.mybir as mybir
    import concourse.tile as tile

    f32 = mybir.dt.float32
    fp16 = mybir.dt.float16
    nc = bacc.Bacc("TRN2", target_bir_lowering=False, debug=False)
    xT = nc.dram_tensor("xTh", [E, N], fp16, kind="ExternalInput").ap()
    w_qkv = nc.dram_tensor("w_qkvh", [E, 3 * E], fp16, kind="ExternalInput").ap()
    w_projh = nc.dram_tensor("w_projh", [E, E], fp16, kind="ExternalInput").ap()
    b_proj = nc.dram_tensor("b_proj", [E], f32, kind="ExternalInput").ap()
    yT = nc.dram_tensor("yT", [E, N], f32, kind="ExternalOutput").ap()

    dbg = None
    if debug_dumps:
        dbg = {
            "vaug0": nc.dram_tensor("d_vaug0", [128, H * DA], fp16,
                                    kind="ExternalOutput").ap(),
            "q0": nc.dram_tensor("d_q0", [128, N], fp16, kind="ExternalOutput").ap(),
            "k0": nc.dram_tensor("d_k0", [128, N], fp16, kind="ExternalOutput").ap(),
            "E000": nc.dram_tensor("d_E000", [128, N], fp16,
                                   kind="ExternalOutput").ap(),
            "acc00": nc.dram_tensor("d_acc00", [128, N], f32,
                                    kind="ExternalOutput").ap(),
            "rb00": nc.dram_tensor("d_rb00", [128, 512], f32,
                                   kind="ExternalOutput").ap(),
            "outT0": nc.dram_tensor("d_outT0", [128, N], fp16,
                                    kind="ExternalOutput").ap(),
        }

    with tile.TileContext(nc) as tc, ExitStack() as ctx:
        pools = (
            ctx.enter_context(tc.tile_pool(name="consts", bufs=1)),
            ctx.enter_context(tc.tile_pool(name="wstr", bufs=12)),
            ctx.enter_context(tc.tile_pool(name="expp", bufs=18)),
            ctx.enter_context(tc.tile_pool(name="qkp", bufs=6)),
            ctx.enter_context(tc.tile_pool(name="rbp", bufs=2)),
            ctx.enter_context(tc.tile_pool(name="ytp", bufs=2)),
            ctx.enter_context(tc.tile_pool(name="psu", bufs=4, space="PSUM")),
        )
        aps = (xT, w_qkv, w_projh, b_proj, yT)
        if loop_n == 1:
            _emit(tc, pools, aps, dbg=dbg)
        else:
            # timing-only path; branch-prefetch hints avoid the per-iteration
            # IRAM refetch stall on the big-body engines
            with tc.For_i(0, loop_n, 1,
                          hint_engines=(mybir.EngineType.PE,
                                        mybir.EngineType.Activation,
                                        mybir.EngineType.DVE)):
                _emit(tc, pools, aps)
    nc.compile()
    return nc


def _get_nc(loop_n=1):
    if loop_n not in _NC_CACHE:
        _NC_CACHE[loop_n] = build_nc(loop_n)
    return _NC_CACHE[loop_n]


def _in_maps(x, w_qkv, w_proj, b_proj):
    x = np.asarray(x, dtype=np.float32)
    w_qkvh = np.ascontiguousarray(
        np.asarray(w_qkv, dtype=np.float32).astype(np.float16))
    w_projh = np.ascontiguousarray(
        np.asarray(w_proj, dtype=np.float32).astype(np.float16))
    b_proj = np.ascontiguousarray(np.asarray(b_proj, dtype=np.float32))
    xTh = np.ascontiguousarray(
        np.transpose(x, (0, 2, 1)).astype(np.float16))  # [B, E, N]
    return [
        {"xTh": xTh[c], "w_qkvh": w_qkvh, "w_projh": w_projh, "b_proj": b_proj}
        for c in range(B)
    ]


def kernel(x, w_qkv, w_proj, b_proj):
    """Full-input entry point: x [8,1024,768] f32 -> out [8,1024,768] f32."""
    from concourse.bass_utils import run_bass_kernel_spmd

    nc = _get_nc()
    in_maps = _in_maps(x, w_qkv, w_proj, b_proj)
    res = run_bass_kernel_spmd(nc, in_maps, core_ids=list(range(B)))
    yT = np.stack([res.results[c]["yT"] for c in range(B)])  # [B, E, N]
    return np.ascontiguousarray(np.transpose(yT, (0, 2, 1)))


# revision 30
# speedup vs baseline: 1.1833x; 1.1833x over previous
"""Trainium2 Bass kernel for nn_Attention_47467978555850.

Multi-head attention (B=8, N=1024, E=768, H=12, D=64), fp32 in/out.
Sharding: data-parallel over batch - one batch element per NeuronCore (8 cores),
no collectives.

All matmul operands are fp16 (host-cast inputs; fp16 mantissa error 2^-11 is
below the fp32r matmul error of the original fp32 pipeline, and every value
here is far inside fp16 range). PSUM accumulation is fp32 throughout. fp16
also enables fast weight loads (FWL) and halves input DMA.

Per-core dataflow (everything stays in "transposed" space so no on-device
transposes are needed; the host transposes/casts x and y, costing no HW time):

  xTh [E, N] fp16 --(w_qkv lhsT-stationary)--> qT, kT fp16 [head-dim major, N]
              (2 heads packed per 128-partition tile)
  xTh (stationary) x w_v (moving) -> v [N, d] -> v_aug fp16 [N, H*128],
              each head block = [v(64) | ones(64)]
  For each head pair f = (2f, 2f+1), for each 512-wide i-chunk c:
    S^T[j,i] both heads : row-packed K=64 matmuls into the two banks of
              one [128,1024] PSUM tile (head A rows 0-63 -> cols 0:512, head B
              rows 64-127 -> cols 512:1024) - concurrent PE row groups
    E = Exp(S^T/8) fp16 : one [128,1024] ScalarE op per (c, j); ScalarE is the
              attention-phase critical engine (~1.07us/op, 96 ops)
    acc[:,0:512]   += vaugA^T @ E[:,0:512]    (8-deep fp16 chains; rows 0-63 =
    acc[:,512:1024]+= vaugB^T @ E[:,512:1024]  out, 64-127 = denominator)
    outT fp16 = acc[0:64] * reciprocal_approx_fast(denominator)
              (the custom DVE op breaks at base_partition 64, so the
              replicated denominator rows are first copied down to 0-63)
  yT = w_projh^T @ outT + b  ->  DMA out as yT [E, N] fp32

PSUM: ONE pool of 4x [128,1024] tiles (8 banks) shared by S / acc / qk-feat /
v / proj so the next pair's feature matmuls can fill PE slack while ScalarE
crunches exp. Next-pair q/k feat tiles are emitted between attention chunks.

Measured absmax-rel error vs fp64: ~4e-4 (gate 2e-2).
"""

import numpy as np

B, N, E = 8, 1024, 768
H, D = 12, 64
NE = E // 128        # 6  e-tiles
NT = N // 128        # 8  token tiles
JT = N // 128        # 8  j tiles (attention context)
CH = N // 512        # 2  512-wide moving chunks
DA = 2 * D           # 128 cols/head in v_aug: [v(64), ones(64)] — the
                     # ones block makes the attn@v matmul replicate the
                     # softmax denominator across 64 psum partitions

_NC_CACHE = {}


def _emit(tc, pools, aps, dbg=None):
    import concourse.mybir as mybir

    nc = tc.nc
    f32 = mybir.dt.float32
    fp16 = mybir.dt.float16
    consts, wstr, expp, qkp, rbp, ytp, psu = pools
    xT, w_qkv, w_projh, b_proj, yT = aps

    def dump(name, src):
        # debug-only: copy an SBUF/PSUM AP out to a DRAM tensor
        if dbg is not None and name in dbg:
            nc.sync.dma_start(out=dbg[name], in_=src)

    def dump_psum(name, src, dt):
        if dbg is not None and name in dbg:
            t = consts.tile(list(src.shape), dt, tag=f"dbg{name}", name=f"dbg{name}")
            nc.vector.tensor_copy(out=t, in_=src)
            nc.sync.dma_start(out=dbg[name], in_=t)

    # ---- persistent SBUF tiles ----
    xt = [consts.tile([128, N], fp16, tag=f"xt{e}", name=f"xt{e}") for e in range(NE)]
    wv = [consts.tile([128, E], fp16, tag=f"wv{e}", name=f"wv{e}") for e in range(NE)]
    b_sb = consts.tile([128, NE], f32, tag="b_sb", name="b_sb")
    vaug = [consts.tile([128, H * DA], fp16, tag=f"va{t}", name=f"va{t}")
            for t in range(NT)]
    outT = [consts.tile([128, N], fp16, tag=f"oT{e}", name=f"oT{e}") for e in range(NE)]

    def load_w_tiles(fcol, fname):
        # all six 128x128 e-blocks of one feature column in a single DMA
        w = wstr.tile([128, E], fp16, tag="w", name=f"w{fname}")
        nc.sync.dma_start(
            out=w.rearrange("p (e c) -> p e c", e=NE),
            in_=w_qkv[:, fcol:fcol + 128].rearrange("(e p) c -> p e c", p=128))
        return [w[:, e * 128:(e + 1) * 128] for e in range(NE)]

    w_q0 = load_w_tiles(0, "q0")
    for e in range(NE):
        nc.sync.dma_start(out=xt[e], in_=xT[e * 128:(e + 1) * 128, :])
    w_k0 = load_w_tiles(E, "k0")
    w_q1 = load_w_tiles(128, "q1")
    w_k1 = load_w_tiles(E + 128, "k1")
    nc.sync.dma_start(out=b_sb, in_=b_proj.rearrange("(t p) -> p t", p=128))


    def load_wv():
        for e in range(NE):
            nc.sync.dma_start(out=wv[e],
                              in_=w_qkv[e * 128:(e + 1) * 128, 2 * E:3 * E])

    # ---- filler queue: generators that emit ~2 matmuls per pull, used to
    # spread feat/proj work into the exp-paced gaps inside attention chunks
    fillq = []

    def fill(n=1):
        for _ in range(n):
            while fillq:
                try:
                    next(fillq[0][1])
                    return
                except StopIteration:
                    fillq.pop(0)
            return

    def drain(upto_pair):
        while fillq and fillq[0][0] <= upto_pair:
            pair, gen = fillq.pop(0)
            for _ in gen:
                pass

    # ---- q/k feature tiles: w_qkv column block stationary, xT moving ----
    def qk_feat_tile(fcol, fname, wts=None):
        dst = qkp.tile([128, N], fp16, tag="qk", name=f"qk{fname}")
        if wts is None:
            wts = load_w_tiles(fcol, fname)

        def gen():
            ps_qk = psu.tile([128, N], f32, tag="ps", name=f"psqk{fname}")
            for c in range(CH):
                cs = slice(c * 512, (c + 1) * 512)
                for e in range(NE):
                    nc.tensor.matmul(
                        out=ps_qk[:, cs], lhsT=(wts[e]), rhs=(xt[e][:, cs]),
                        start=(e == 0), stop=(e == NE - 1),
                    )
                    if e % 2 == 1:
                        yield
                nc.vector.tensor_copy(out=dst[:, cs], in_=ps_qk[:, cs])
                yield
        return dst, gen()

    # ---- v = x @ w_v  (xT tiles stationary, w_v moving) -> vaug bf16 ----
    def emit_v_phase():
        for t in range(NT):
            ps_v = psu.tile([128, N], f32, tag="ps", name=f"psv{t}")
            for (c0, cl) in ((0, 512), (512, 256)):
                for e in range(NE):
                    nc.tensor.matmul(
                        out=ps_v[:, c0:c0 + cl],
                        lhsT=(xt[e][:, t * 128:(t + 1) * 128]),
                        rhs=(wv[e][:, c0:c0 + cl]),
                        start=(e == 0), stop=(e == NE - 1),
                    )
            va3 = vaug[t].rearrange("p (h c) -> p h c", h=H)
            nc.vector.tensor_copy(
                out=va3[:, :, 0:D],
                in_=ps_v[:, 0:E].rearrange("p (h c) -> p h c", h=H),
            )
            nc.vector.memset(va3[:, :, D:DA], 1.0)
            if t == 0:
                dump("vaug0", vaug[0])

    # ---- attention for head pair f, one 512-wide i-chunk ----
    def attention_chunk(f, c, qTf, kTf, flat=None):
        hA, hB = 2 * f, 2 * f + 1
        cs = slice(c * 512, (c + 1) * 512)

        def mm2exp(j):
            js = slice(j * 128, (j + 1) * 128)
            S = psu.tile([128, N], f32, tag="ps", name=f"S{f}_{c}_{j}")
            for pb, col0 in ((0, 0), (64, 512)):
                nc.tensor.matmul(
                    out=S[:, col0:col0 + 512],
                    lhsT=kTf[pb:pb + 64, js],
                    rhs=qTf[pb:pb + 64, cs],
                    start=True, stop=True,
                )
            Ej = expp.tile([128, N], fp16, tag="e", name=f"E{f}_{c}_{j}")
            nc.scalar.activation(
                out=Ej, in_=S,
                func=mybir.ActivationFunctionType.Exp, scale=0.125)
            if f == 0 and c == 0 and j == 0:
                dump("E000", Ej)
            return Ej

        def mm3(j, Ej):
            for col0, h in ((0, hA), (512, hB)):
                nc.tensor.matmul(
                    out=acc[:, col0:col0 + 512],
                    lhsT=(vaug[j][:, h * DA:(h + 1) * DA]),
                    rhs=(Ej[:, col0:col0 + 512]),
                    start=(j == 0), stop=(j == JT - 1),
                )

        if flat == "scores":
            # emit only the score/exp stream; mm3 chains deferred (pair 0)
            Es = [mm2exp(j) for j in range(JT)]
            return Es
        if flat == "mm3":
            acc = psu.tile([128, N], f32, tag="ps", name=f"acc{f}_{c}")
            for j in range(JT):
                fill()
                mm3(j, qTf[j])           # qTf carries the E list here
        else:
            E_cur = mm2exp(0)
            acc = psu.tile([128, N], f32, tag="ps", name=f"acc{f}_{c}")
            for j in range(JT):
                E_next = mm2exp(j + 1) if j + 1 < JT else None
                fill()
                mm3(j, E_cur)
                E_cur = E_next

        if f == 0 and c == 0:
            dump_psum("acc00", acc, f32)
        # custom-DVE ops misbehave at base_partition 64: stage the replicated
        # denominators down to partitions 0-63 with a native copy first
        den = rbp.tile([128, N], f32, tag="den", name=f"den{f}_{c}")
        rb = rbp.tile([128, N], f32, tag="rb", name=f"rb{f}_{c}")
        nc.vector.tensor_copy(out=den[0:64, :], in_=acc[64:128, :])
        nc.vector.reciprocal_approx_fast(out=rb[0:64, :], in_=den[0:64, :])
        if f == 0 and c == 0:
            dump("rb00", rb[:, 0:512])
        for col0, h in ((0, hA), (512, hB)):
            pb = (h % 2) * 64
            nc.vector.tensor_mul(outT[f][pb:pb + 64, cs],
                                 acc[0:64, col0:col0 + 512],
                                 rb[0:64, col0:col0 + 512])

    def load_proj_w(g):
        wg = wstr.tile([128, E], fp16, tag="w", name=f"wp{g}")
        nc.sync.dma_start(
            out=wg.rearrange("p (e c) -> p e c", e=NE),
            in_=w_projh[:, g * 128:(g + 1) * 128].rearrange(
                "(e p) c -> p e c", p=128))
        return wg

    # Tile dependencies follow EMISSION order, so a proj matmul reading
    # outT[e] must be emitted after the normalization that writes it.  The
    # e<=4 head of proj(0) can therefore run as filler inside pair 5 (outT[4]
    # is complete once pair 4's norms are emitted), but every e=5 matmul must
    # be emitted after pair 5.
    proj_ps = {}

    def proj_head_gen(g, wg):
        wts = [wg[:, e * 128:(e + 1) * 128] for e in range(NE)]

        def gen():
            ps_y = psu.tile([128, N], f32, tag="ps", name=f"psy{g}")
            proj_ps[g] = ps_y
            for e in range(NE - 1):
                for c in range(CH):
                    cs = slice(c * 512, (c + 1) * 512)
                    nc.tensor.matmul(
                        out=ps_y[:, cs], lhsT=(wts[e]), rhs=(outT[e][:, cs]),
                        start=(e == 0), stop=False,
                    )
                yield
        return gen()

    def proj_finish(g, wg):
        ps_y = proj_ps[g]
        for c in range(CH):
            cs = slice(c * 512, (c + 1) * 512)
            nc.tensor.matmul(
                out=ps_y[:, cs], lhsT=(wg[:, (NE - 1) * 128:NE * 128]),
                rhs=(outT[NE - 1][:, cs]), start=False, stop=True,
            )
        yt = ytp.tile([128, N], f32, tag="yt", name=f"yt{g}")
        nc.scalar.activation(out=yt, in_=ps_y,
                             func=mybir.ActivationFunctionType.Identity,
                             bias=b_sb[:, g:g + 1], scale=1.0)
        nc.sync.dma_start(out=yT[g * 128:(g + 1) * 128, :], in_=yt)

    # ---- main schedule ----
    # prologue: feats for pairs 0 and 1 emitted eagerly (weight tiles were
    # preloaded before the bulk xt DMA).  Pair 0 emits BOTH chunks' score/exp
    # streams back-to-back so ScalarE runs 16 exps while the PE does the whole
    # v phase underneath; its attn@v chains follow the v phase.  From pair 1
    # on, feat tiles for pair f+2 are built as fillers pulled between the
    # j-steps (2 matmuls per pull), so they run inside the exp-paced dribbles
    # instead of as a block at the pair boundary.  The first two proj tiles
    # join the filler queue during pair 5.
    qk = {}
    for fcol, fname, wts in ((0, "q0", w_q0), (E, "k0", w_k0),
                             (128, "q1", w_q1), (E + 128, "k1", w_k1)):
        dst, g = qk_feat_tile(fcol, fname, wts)
        for _ in g:
            pass
        qk.setdefault(fname[1:], []).append(dst)
    qk = {0: tuple(qk["0"]), 1: tuple(qk["1"])}
    dump("q0", qk[0][0])
    dump("k0", qk[0][1])
    load_wv()

    for f in range(NE):
        drain(f)
        qTf, kTf = qk.pop(f)
        if f == 0:
            Es0 = attention_chunk(0, 0, qTf, kTf, flat="scores")
            Es1 = attention_chunk(0, 1, qTf, kTf, flat="scores")
            emit_v_phase()
            attention_chunk(0, 0, Es0, None, flat="mm3")
            attention_chunk(0, 1, Es1, None, flat="mm3")
        else:
            attention_chunk(f, 0, qTf, kTf)
            attention_chunk(f, 1, qTf, kTf)
        if f + 2 < NE:
            dq, gq = qk_feat_tile((f + 2) * 128, f"q{f + 2}")
            dk, gk = qk_feat_tile(E + (f + 2) * 128, f"k{f + 2}")
            qk[f + 2] = (dq, dk)
            fillq.append((f + 2, gq))
            fillq.append((f + 2, gk))
        if f == NE - 3:
            proj_w = [load_proj_w(g) for g in range(NE)]
        if f == NE - 2:
            fillq.append((99, proj_head_gen(0, proj_w[0])))

    dump("outT0", outT[0])

    # ---- proj: yT = w_projh^T @ outT + b  (fp16 x fp16) ----
    drain(99)
    proj_finish(0, proj_w[0])
    for g in range(1, NE):
        for _ in proj_head_gen(g, proj_w[g]):
            pass
        proj_finish(g, proj_w[g])


def build_nc(loop_n=1, debug_dumps=False):
    """Build + compile the per-core Bass program. loop_n>1 wraps the body in a
    dynamic loop (used only for timing runs)."""
    from contextlib import ExitStack
    import concourse.bacc as bacc
    import concourse.mybir as mybir
    import concourse.tile as tile

    f32 = mybir.dt.float32
    fp16 = mybir.dt.float16
    nc = bacc.Bacc("TRN2", target_bir_lowering=False, debug=False)
    xT = nc.dram_tensor("xTh", [E, N], fp16, kind="ExternalInput").ap()
    w_qkv = nc.dram_tensor("w_qkvh", [E, 3 * E], fp16, kind="ExternalInput").ap()
    w_projh = nc.dram_tensor("w_projh", [E, E], fp16, kind="ExternalInput").ap()
    b_proj = nc.dram_tensor("b_proj", [E], f32, kind="ExternalInput").ap()
    yT = nc.dram_tensor("yT", [E, N], f32, kind="ExternalOutput").ap()

    dbg = None
    if debug_dumps:
        dbg = {
            "vaug0": nc.dram_tensor("d_vaug0", [128, H * DA], fp16,
                                    kind="ExternalOutput").ap(),
            "q0": nc.dram_tensor("d_q0", [128, N], fp16, kind="ExternalOutput").ap(),
            "k0": nc.dram_tensor("d_k0", [128, N], fp16, kind="ExternalOutput").ap(),
            "E000": nc.dram_tensor("d_E000", [128, N], fp16,
                                   kind="ExternalOutput").ap(),
            "acc00": nc.dram_tensor("d_acc00", [128, N], f32,
                                    kind="ExternalOutput").ap(),
            "rb00": nc.dram_tensor("d_rb00", [128, 512], f32,
                                   kind="ExternalOutput").ap(),
            "outT0": nc.dram_tensor("d_outT0", [128, N], fp16,
                                    kind="ExternalOutput").ap(),
        }

    with tile.TileContext(nc) as tc, ExitStack() as ctx:
        pools = (
            ctx.enter_context(tc.tile_pool(name="consts", bufs=1)),
            ctx.enter_context(tc.tile_pool(name="wstr", bufs=12)),
            ctx.enter_context(tc.tile_pool(name="expp", bufs=18)),
            ctx.enter_context(tc.tile_pool(name="qkp", bufs=6)),
            ctx.enter_context(tc.tile_pool(name="rbp", bufs=2)),
            ctx.enter_context(tc.tile_pool(name="ytp", bufs=4)),
            ctx.enter_context(tc.tile_pool(name="psu", bufs=4, space="PSUM")),
        )
        aps = (xT, w_qkv, w_projh, b_proj, yT)
        if loop_n == 1:
            _emit(tc, pools, aps, dbg=dbg)
        else:
            # timing-only path; branch-prefetch hints avoid the per-iteration
            # IRAM refetch stall on the big-body engines
            with tc.For_i(0, loop_n, 1,
                          hint_engines=(mybir.EngineType.PE,
                                        mybir.EngineType.Activation,
                                        mybir.EngineType.DVE)):
                _emit(tc, pools, aps)
    nc.compile()
    return nc


def _get_nc(loop_n=1):
    if loop_n not in _NC_CACHE:
        _NC_CACHE[loop_n] = build_nc(loop_n)
    return _NC_CACHE[loop_n]


def _in_maps(x, w_qkv, w_proj, b_proj):
    x = np.asarray(x, dtype=np.float32)
    w_qkvh = np.ascontiguousarray(
        np.asarray(w_qkv, dtype=np.float32).astype(np.float16))
    w_projh = np.ascontiguousarray(
        np.asarray(w_proj, dtype=np.float32).astype(np.float16))
    b_proj = np.ascontiguousarray(np.asarray(b_proj, dtype=np.float32))
    xTh = np.ascontiguousarray(
        np.transpose(x, (0, 2, 1)).astype(np.float16))  # [B, E, N]
    return [
        {"xTh": xTh[c], "w_qkvh": w_qkvh, "w_projh": w_projh, "b_proj": b_proj}
        for c in range(B)
    ]


def kernel(x, w_qkv, w_proj, b_proj):
    """Full-input entry point: x [8,1024,768] f32 -> out [8,1024,768] f32."""
    from concourse.bass_utils import run_bass_kernel_spmd

    nc = _get_nc()
    in_maps = _in_maps(x, w_qkv, w_proj, b_proj)
    res = run_bass_kernel_spmd(nc, in_maps, core_ids=list(range(B)))
    yT = np.stack([res.results[c]["yT"] for c in range(B)])  # [B, E, N]
    return np.ascontiguousarray(np.transpose(yT, (0, 2, 1)))
